# revision 1
# baseline (speedup 1.0000x reference)
"""Multi-head self-attention Trainium2 kernel (Bass/Tile), batch-parallel
over 8 NeuronCores.

Problem (hardcoded): B=8, L=1024, D=1024, H=16, hd=64, f32.
  qkv = x @ w_qkv + b_qkv ; per-head scores = q k^T / 8 ; mask ; softmax ;
  out = (P v) heads-merged @ w_out + b_out.

Sharding: one batch element per core (data parallel); full weights on every
core. No collectives.

Per-core dataflow (all real matmuls in float32r, N=512, f32 PSUM accum):
  - host provides xT (D x L, dim-major) and w_qkv pre-blocked so every DMA is
    contiguous.
  - qkvT[3D x L] = w_qkv^T @ x^T via 24 M-tiles x 8 K-chunks (PSUM accum),
    evacuated with per-partition b_qkv add (DVE) -> SBUF f32r.
  - per head: ST = k^T q (scores TRANSPOSED: [Lk x Lq]) -> exp via ScalarE
    with scale=1/8 and per-partition mask bias (exact masking for free),
    no max-subtraction (inputs bounded; softmax is shift-invariant).
  - V^T per head via PE transposes into one PSUM bank, augmented with a ones
    column so the attention matmul also produces the softmax denominator.
  - O_un^T[65 x Lq] = V'^T E accumulated over Lk chunks. Rows 0:64 are the
    unnormalized head output (dim-major), row 64 the denominator.
  - normalization deferred: OT chunk tiles [128 x L] collect 2 heads' raw
    rows; R = 1/denom broadcast across partitions via SBUF->SBUF DMA;
    one DVE multiply normalizes and rounds to f32r.
  - final = OT^T @ w_out per Lq-tile (+ b_out broadcast add) -> token-major
    output, DMA'd straight to DRAM.
"""

import sys

import numpy as np

try:
    import concourse.bass as bass  # noqa: F401
except Exception:  # pragma: no cover - defensive path setup
    for p in ("/opt/trn_rl_repo", "/opt/pypackages"):
        if p not in sys.path:
            sys.path.insert(0, p)
    import concourse.bass as bass  # noqa: F401

from contextlib import ExitStack

import concourse.tile as tile
from concourse import bacc, mybir
from concourse.bass_utils import run_bass_kernel_spmd
from concourse.masks import make_identity

F32 = mybir.dt.float32
F32R = mybir.dt.float32r

B, L, D = 8, 1024, 1024
H, HD = 16, 64
D3 = 3 * D
N_CORES = 8
PART = 128
NK = D // PART  # 8 contraction chunks
NM = D3 // PART  # 24 qkv output tiles
NLQ = L // PART  # 8 query tiles
NLK = L // PART  # 8 key tiles
MG = 3  # qkv M-tiles per PSUM group


def build_nc(debug=False):
    nc = bacc.Bacc("TRN2", target_bir_lowering=False, debug=False)

    xT = nc.dram_tensor("xT", (D, L), F32, kind="ExternalInput").ap()
    # w_qkv blocked on host: wqkv_blk[m, p, k, c] = w_qkv[k*128 + p, m*128 + c]
    wqkv_blk = nc.dram_tensor(
        "wqkv_blk", (NM, PART, NK, PART), F32, kind="ExternalInput"
    ).ap()
    bqkv = nc.dram_tensor("bqkv", (D3,), F32, kind="ExternalInput").ap()
    wout = nc.dram_tensor("wout", (D, D), F32, kind="ExternalInput").ap()
    bout = nc.dram_tensor("bout", (PART, D), F32, kind="ExternalInput").ap()
    maskb = nc.dram_tensor("maskb", (L,), F32, kind="ExternalInput").ap()
    sel = nc.dram_tensor("sel", (2, PART), F32, kind="ExternalInput").ap()
    Y = nc.dram_tensor("Y", (L, D), F32, kind="ExternalOutput").ap()
    dbg = {}
    if debug:
        for nm, shp in [
            ("dbg_q", (PART, L)), ("dbg_k", (PART, L)), ("dbg_v", (PART, L)),
            ("dbg_e", (PART, L)), ("dbg_vt", (PART, NLK * (HD + 1))),
            ("dbg_po", (PART, L)), ("dbg_rt", (PART, L)), ("dbg_ot", (PART, L)),
        ]:
            dbg[nm] = nc.dram_tensor(nm, shp, F32, kind="ExternalOutput").ap()

    with tile.TileContext(nc) as tc, ExitStack() as ctx:
        singles = ctx.enter_context(tc.tile_pool(name="singles", bufs=1))

        ident = singles.tile([PART, PART], F32)
        make_identity(nc, ident[:])
        ones_sb = singles.tile([PART, 1], F32)
        nc.vector.memset(ones_sb[:], 1.0)
        zeros_sb = singles.tile([PART, 1], F32)
        nc.vector.memset(zeros_sb[:], 0.0)
        sel_sb = singles.tile([2, PART], F32R)
        nc.sync.dma_start(sel_sb[:], sel[:, :].bitcast(F32R))
        bqkv_sb = singles.tile([PART, NM], F32)
        nc.sync.dma_start(bqkv_sb[:], bqkv.rearrange("(c p) -> p c", p=PART))
        mb_sb = singles.tile([PART, NLK], F32)
        nc.sync.dma_start(mb_sb[:], maskb.rearrange("(c p) -> p c", p=PART))

        # ---- tiles that must survive across phases ----
        qkvT_pool = ctx.enter_context(tc.tile_pool(name="qkvT", bufs=1))
        qkvT = []
        for m in range(NM):
            t = qkvT_pool.tile([PART, L], F32R, tag=f"qkvT{m}")
            qkvT.append(t)

        ot_pool = ctx.enter_context(tc.tile_pool(name="otpool", bufs=1))
        ot_fin = []
        for j in range(NK):
            t = ot_pool.tile([PART, L], F32R, tag=f"ot{j}")
            ot_fin.append(t)

        # ================= phase 1: qkv projection =================
        with (
            tc.tile_pool(name="xt", bufs=1) as xt_pool,
            tc.tile_pool(name="wblk", bufs=2 * MG) as wblk_pool,
            tc.tile_pool(name="pq", bufs=MG, space="PSUM") as pq_pool,
        ):
            xt = []
            for k in range(NK):
                t = xt_pool.tile([PART, L], F32R, tag=f"xt{k}")
                xt.append(t)

            def load_xt(k):
                nc.sync.dma_start(
                    xt[k][:], xT[k * PART : (k + 1) * PART, :].bitcast(F32R)
                )

            xt_loaded = 0
            for g in range(NM // MG):
                ms = [g * MG + i for i in range(MG)]
                wtiles = {}
                for m in ms:
                    wt = wblk_pool.tile([PART, NK * PART], F32R, tag="wblk")
                    src = wqkv_blk[m].rearrange("p k c -> p (k c)").bitcast(F32R)
                    hw = NK * PART // 2
                    nc.sync.dma_start(wt[:, 0:hw], src[:, 0:hw])
                    nc.sync.dma_start(wt[:, hw : 2 * hw], src[:, hw : 2 * hw])
                    wtiles[m] = wt
                pts = {}
                for m in ms:
                    pt = pq_pool.tile([PART, L], F32, tag="pq")
                    pts[m] = pt
                for k in range(NK):
                    while xt_loaded < min(NK, k + 2) and (g > 0 or True):
                        load_xt(xt_loaded)
                        xt_loaded += 1
                    for m in ms:
                        for nh in range(2):
                            nc.tensor.matmul(
                                pts[m][:, nh * 512 : (nh + 1) * 512],
                                wtiles[m][:, k * PART : (k + 1) * PART],
                                xt[k][:, nh * 512 : (nh + 1) * 512],
                                start=(k == 0),
                                stop=(k == NK - 1),
                            )
                for m in ms:
                    nc.scalar.activation(
                        qkvT[m][:],
                        pts[m][:],
                        mybir.ActivationFunctionType.Identity,
                        bias=bqkv_sb[:, m : m + 1],
                        scale=1.0,
                    )
            if debug:
                nc.sync.dma_start(dbg["dbg_q"][:, :], qkvT[0][:].bitcast(F32))
                nc.sync.dma_start(dbg["dbg_k"][:, :], qkvT[NLQ][:].bitcast(F32))
                nc.sync.dma_start(dbg["dbg_v"][:, :], qkvT[2 * NLQ][:].bitcast(F32))

        # ================= phase 2: attention per head =================
        with (
            tc.tile_pool(name="epool", bufs=8) as e_pool,
            tc.tile_pool(name="vtpool", bufs=3) as vt_pool,
            tc.tile_pool(name="kzpool", bufs=2) as kz_pool,
            tc.tile_pool(name="otraw", bufs=3) as otraw_pool,
            tc.tile_pool(name="rcp", bufs=1) as rcp_pool,
            tc.tile_pool(name="rc2p", bufs=3) as rc2_pool,
            tc.tile_pool(name="pst", bufs=2, space="PSUM") as pst_pool,
            tc.tile_pool(name="po", bufs=2, space="PSUM") as po_pool,
        ):
            def emit_vt_pair(j):
                """PE-transpose the full V pair tile (K=128) once; split the
                transposed chunks into the two heads' [128, 8*65] vt tiles
                (ones column appended for the denominator row)."""
                vsrc = qkvT[2 * NLQ + j]
                pvt = pst_pool.tile([PART, L], F32, tag="pst")
                for c in range(NLK):
                    nc.tensor.transpose(
                        pvt[:, c * PART : (c + 1) * PART],
                        vsrc[:, c * PART : (c + 1) * PART].bitcast(F32),
                        ident[:],
                    )
                pvt3 = pvt[:].rearrange("p (c w) -> p c w", w=PART)
                out = []
                for side in range(2):
                    vt = vt_pool.tile([PART, NLK * (HD + 1)], F32R, tag="vt")
                    vt3 = vt[:].rearrange("p (c w) -> p c w", w=HD + 1)
                    nc.vector.tensor_copy(
                        vt3[:, :, 0:HD],
                        pvt3[:, :, side * HD : (side + 1) * HD],
                    )
                    for c in range(NLK):
                        nc.vector.tensor_copy(
                            vt[:, c * (HD + 1) + HD : (c + 1) * (HD + 1)],
                            ones_sb[:],
                        )
                    out.append(vt)
                return out

            def emit_kz(h):
                """k operand for head h zero-padded to K=128 (the sibling
                head's rows are zeroed so cross-terms vanish)."""
                j = h // 2
                kt = qkvT[NLQ + j]
                ro = (h % 2) * HD
                ro2 = HD - ro  # complement offset
                kz = kz_pool.tile([PART, L], F32R, tag="kz")
                nc.vector.tensor_copy(kz[ro : ro + HD, :], kt[ro : ro + HD, :])
                nc.vector.tensor_copy(
                    kz[ro2 : ro2 + HD, :],
                    zeros_sb[ro2 : ro2 + HD, 0:1].to_broadcast((HD, L)),
                )
                return kz

            def emit_score_chunk(h, c, kz):
                """ST chunk c of head h (2 matmuls, K=128) + fused exp."""
                j = h // 2
                qt = qkvT[j]
                st = pst_pool.tile([PART, L], F32, tag="pst")
                for nh in range(2):
                    nc.tensor.matmul(
                        st[:, nh * 512 : (nh + 1) * 512],
                        kz[:, c * PART : (c + 1) * PART],
                        qt[:, nh * 512 : (nh + 1) * 512],
                        start=True,
                        stop=True,
                    )
                et = e_pool.tile([PART, L], F32R, tag="e")
                nc.scalar.activation(
                    et[:],
                    st[:],
                    mybir.ActivationFunctionType.Exp,
                    bias=mb_sb[:, c : c + 1],
                    scale=1.0 / 8.0,
                )
                return et

            def flush_pending():
                pj, potr, prc2 = pending.pop(0)
                rt = po_pool.tile([PART, L], F32, tag="po")
                for half in range(2):
                    ns = slice(half * 512, (half + 1) * 512)
                    nc.tensor.matmul(
                        rt[:, ns], sel_sb[:], prc2[0:2, ns],
                        start=True, stop=True,
                    )
                nc.vector.tensor_mul(ot_fin[pj][:], potr[:], rt[:])

            pending = []
            otr = None
            rc2 = None
            hop = None
            # prologue: head 0 scores run alone
            vts = {}
            vts[0], vts[1] = emit_vt_pair(0)
            kzs = {0: emit_kz(0)}
            ets = {0: [emit_score_chunk(0, c, kzs[0]) for c in range(NLK)]}

            for h in range(H):
                j = h // 2
                ro = (h % 2) * HD
                if h + 1 < H:
                    if (h + 1) % 2 == 0:
                        vts[h + 1], vts[h + 2] = emit_vt_pair((h + 1) // 2)
                    kzs[h + 1] = emit_kz(h + 1)
                    ets[h + 1] = []
                vt = vts.pop(h)
                kzs.pop(h - 1, None)
                etiles = ets.pop(h)

                # interleaved: next head's score chunk + this head's attn.V
                po = po_pool.tile([PART, L], F32, tag="po")
                for c in range(NLK):
                    if h + 1 < H:
                        ets[h + 1].append(emit_score_chunk(h + 1, c, kzs[h + 1]))
                    for nh in range(2):
                        nc.tensor.matmul(
                            po[0 : HD + 1, nh * 512 : (nh + 1) * 512],
                            vt[:, c * (HD + 1) : (c + 1) * (HD + 1)],
                            etiles[c][:, nh * 512 : (nh + 1) * 512],
                            start=(c == 0),
                            stop=(c == NLK - 1),
                        )

                if len(pending) >= 2:
                    flush_pending()

                if h % 2 == 0:
                    otr = otraw_pool.tile([PART, L], F32, tag="otraw")
                    rc2 = rc2_pool.tile([2, L], F32R, tag="rc2")
                with nc.allow_low_precision(reason="f32r denom reciprocal"):
                    if h % 2 == 0:
                        nc.vector.reciprocal(rc2[0:1, :], po[HD : HD + 1, :])
                    else:
                        hop = rcp_pool.tile([1, L], F32R, tag="rcp")
                        nc.vector.reciprocal(hop[:], po[HD : HD + 1, :])
                nc.vector.tensor_copy(otr[ro : ro + HD, :], po[0:HD, :])
                if h % 2 == 1:
                    nc.sync.dma_start(rc2[1:2, :], hop[0:1, :])
                    pending.append((j, otr, rc2))

            while pending:
                flush_pending()

        # ================= phase 3: output projection =================
        with (
            tc.tile_pool(name="woutp", bufs=1) as wout_pool,
            tc.tile_pool(name="fsb", bufs=2) as f_pool,
            tc.tile_pool(name="pf", bufs=2, space="PSUM") as pf_pool,
        ):
            bout_sb = wout_pool.tile([PART, D], F32, tag="bout")
            nc.sync.dma_start(bout_sb[:], bout[:, :])
            wo = []
            for k in range(NK):
                t = wout_pool.tile([PART, D], F32R, tag=f"wo{k}")
                nc.sync.dma_start(
                    t[:], wout[k * PART : (k + 1) * PART, :].bitcast(F32R)
                )
                wo.append(t)
            for lq in range(NLQ):
                pf = pf_pool.tile([PART, D], F32, tag="pf")
                for k in range(NK):
                    for nh in range(2):
                        nc.tensor.matmul(
                            pf[:, nh * 512 : (nh + 1) * 512],
                            ot_fin[k][:, lq * PART : (lq + 1) * PART],
                            wo[k][:, nh * 512 : (nh + 1) * 512],
                            start=(k == 0),
                            stop=(k == NK - 1),
                        )
                fs = f_pool.tile([PART, D], F32, tag="fsb")
                for half in range(2):
                    ns = slice(half * 512, (half + 1) * 512)
                    nc.vector.tensor_add(fs[:, ns], pf[:, ns], bout_sb[:, ns])
                    nc.sync.dma_start(Y[lq * PART : (lq + 1) * PART, ns], fs[:, ns])

    nc.compile()
    return nc


_NC_CACHE = None


def _get_nc():
    global _NC_CACHE
    if _NC_CACHE is None:
        _NC_CACHE = build_nc()
    return _NC_CACHE


def make_in_maps(x, attn_mask, w_qkv, b_qkv, w_out, b_out):
    """Host-side sharding + layout prep -> per-core input maps."""
    x = np.asarray(x, dtype=np.float32)
    attn_mask = np.asarray(attn_mask)
    w_qkv = np.asarray(w_qkv, dtype=np.float32)
    b_qkv = np.ascontiguousarray(np.asarray(b_qkv, dtype=np.float32))
    w_out = np.ascontiguousarray(np.asarray(w_out, dtype=np.float32))
    b_out = np.asarray(b_out, dtype=np.float32)

    # wqkv_blk[m, p, k, c] = w_qkv[k*128 + p, m*128 + c]
    wblk = np.ascontiguousarray(
        w_qkv.reshape(NK, PART, NM, PART).transpose(2, 1, 0, 3)
    )
    maskbias = np.where(attn_mask.astype(bool), 0.0, -10000.0).astype(np.float32)

    sel_host = np.zeros((2, PART), dtype=np.float32)
    sel_host[0, 0:HD] = 1.0
    sel_host[1, HD:PART] = 1.0
    in_maps = []
    for b in range(B):
        in_maps.append(
            {
                "xT": np.ascontiguousarray(x[b].T),
                "wqkv_blk": wblk,
                "bqkv": b_qkv,
                "wout": w_out,
                "bout": np.ascontiguousarray(np.broadcast_to(b_out, (PART, D))),
                "maskb": np.ascontiguousarray(maskbias[b]),
                "sel": sel_host,
            }
        )
    return in_maps


def kernel(x, attn_mask, w_qkv, b_qkv, w_out, b_out):
    in_maps = make_in_maps(x, attn_mask, w_qkv, b_qkv, w_out, b_out)
    nc = _get_nc()
    res = run_bass_kernel_spmd(nc, in_maps, core_ids=list(range(N_CORES)))
    return np.stack([res.results[b]["Y"] for b in range(B)], axis=0)


if __name__ == "__main__":
    rng = np.random.default_rng(0)
    inputs = {
        "x": rng.standard_normal((B, L, D), dtype=np.float32),
        "attn_mask": np.ones((B, L), dtype=bool),
        "w_qkv": ((rng.random((D, D3), dtype=np.float32) - 0.5) / 16.0),
        "b_qkv": np.zeros((D3,), dtype=np.float32),
        "w_out": ((rng.random((D, D), dtype=np.float32) - 0.5) / 16.0),
        "b_out": np.zeros((D,), dtype=np.float32),
    }
    y = kernel(**inputs)
    print(y.shape, y.dtype)



# revision 11
# speedup vs baseline: 1.4100x; 1.4100x over previous
"""Multi-head self-attention Trainium2 kernel (Bass/Tile), batch-parallel
over 8 NeuronCores.

Problem (hardcoded): B=8, L=1024, D=1024, H=16, hd=64, f32 in/out.
  qkv = x @ w_qkv + b_qkv ; per-head scores = q k^T / 8 ; mask ; softmax ;
  out = (P v) heads-merged @ w_out + b_out.

Sharding: one batch element per core (data parallel); full weights on every
core. No collectives.

v2 dataflow (bf16 matmul path, f32 PSUM accumulation):
  - host ships xT, w_qkv (blocked), w_out in bf16; biases/mask in f32.
  - phase 1: qkvT[3D x L] = w_qkv^T @ x^T, 24 M-tiles x 8 K-chunks, PSUM
    accum, ScalarE evacuation with per-partition b_qkv add -> bf16 SBUF.
  - phase 2 per head pair (j = 0..7):
      * V^T via 8 PE transposes -> PSUM; DVE-copied into per-head vt tiles
        [128 x 8*66] with a ones column per chunk (softmax denominator row).
      * scores: both sibling heads' ST chunks [128k x 1024q] run row-packed
        (K=64, tile_position (0,0)/(64,0)) -> concurrent in the PE array.
      * exp via ScalarE with scale=1/8 and per-partition mask bias -> bf16.
      * attn.V: po[65 x 512] tiles accumulate V'^T E over k chunks; row 64
        is the softmax denominator.
      * denominators DMA'd into a [2 x 1024] tile; reciprocal_approx_fast
        (single DVE op) replaces the former 8 us/head serial reciprocal.
      * sel-matmul broadcasts 1/denom across partitions; one DVE multiply
        normalizes po -> ot_fin bf16 (deferred one pair to keep PE busy).
  - phase 3: Y = ot^T @ w_out per Lq-tile + b_out broadcast add -> f32 DRAM.
"""

import sys

import numpy as np

try:
    import concourse.bass as bass  # noqa: F401
except Exception:  # pragma: no cover - defensive path setup
    for p in ("/opt/trn_rl_repo", "/opt/pypackages"):
        if p not in sys.path:
            sys.path.insert(0, p)
    import concourse.bass as bass  # noqa: F401

from contextlib import ExitStack

import ml_dtypes

import concourse.tile as tile
from concourse import bacc, mybir
from concourse.bass_utils import run_bass_kernel_spmd
from concourse.masks import make_identity

F32 = mybir.dt.float32
F32R = mybir.dt.float32r
BF16 = mybir.dt.bfloat16

B, L, D = 8, 1024, 1024
H, HD = 16, 64
D3 = 3 * D
N_CORES = 8
PART = 128
NK = D // PART  # 8 contraction chunks
NM = D3 // PART  # 24 qkv output tiles
NLQ = L // PART  # 8 query tiles
NLK = L // PART  # 8 key tiles
MG = 3  # qkv M-tiles per PSUM group
VW = HD + 2  # 66: V chunk width in vt tiles (64 dims + ones col + pad)


def build_nc(debug=False):
    nc = bacc.Bacc("TRN2", target_bir_lowering=False, debug=False)

    xT = nc.dram_tensor("xT", (D, L), BF16, kind="ExternalInput").ap()
    # w_qkv blocked on host: wqkv_blk[m, p, k, c] = w_qkv[k*128 + p, m*128 + c]
    wqkv_blk = nc.dram_tensor(
        "wqkv_blk", (NM, PART, NK, PART), BF16, kind="ExternalInput"
    ).ap()
    bqkv = nc.dram_tensor("bqkv", (D3,), F32, kind="ExternalInput").ap()
    wout = nc.dram_tensor("wout", (D, D), BF16, kind="ExternalInput").ap()
    bout = nc.dram_tensor("bout", (PART, D), BF16, kind="ExternalInput").ap()
    maskb = nc.dram_tensor("maskb", (L,), F32, kind="ExternalInput").ap()
    sel = nc.dram_tensor("sel", (2, PART), F32, kind="ExternalInput").ap()
    Y = nc.dram_tensor("Y", (L, D), F32, kind="ExternalOutput").ap()

    with tile.TileContext(nc) as tc, ExitStack() as ctx:
        singles = ctx.enter_context(tc.tile_pool(name="singles", bufs=1))

        ident = singles.tile([PART, PART], BF16)
        make_identity(nc, ident[:])
        sel_sb = singles.tile([2, PART], F32R)
        nc.sync.dma_start(sel_sb[:], sel[:, :].bitcast(F32R))
        bqkv_sb = singles.tile([PART, NM], F32)
        nc.sync.dma_start(bqkv_sb[:], bqkv.rearrange("(c p) -> p c", p=PART))
        mb_sb = singles.tile([PART, NLK], F32)
        nc.sync.dma_start(mb_sb[:], maskb.rearrange("(c p) -> p c", p=PART))

        # ---- tiles that must survive across phases ----
        qkvT_pool = ctx.enter_context(tc.tile_pool(name="qkvT", bufs=1))
        qkvT = []
        for m in range(NM):
            t = qkvT_pool.tile([PART, L], BF16, tag=f"qkvT{m}")
            qkvT.append(t)

        ot_pool = ctx.enter_context(tc.tile_pool(name="otpool", bufs=1))
        ot_fin = []
        for j in range(NK):
            t = ot_pool.tile([PART, L], BF16, tag=f"ot{j}")
            ot_fin.append(t)

        # ================= phase 1: qkv projection =================
        with (
            tc.tile_pool(name="xt", bufs=1) as xt_pool,
            tc.tile_pool(name="wblk", bufs=2 * MG) as wblk_pool,
            tc.tile_pool(name="pq", bufs=MG, space="PSUM") as pq_pool,
        ):
            xt = []
            for k in range(NK):
                t = xt_pool.tile([PART, L], BF16, tag=f"xt{k}")
                xt.append(t)

            def load_xt(k):
                nc.sync.dma_start(xt[k][:], xT[k * PART : (k + 1) * PART, :])

            xt_loaded = 0
            for g in range(NM // MG):
                ms = [g * MG + i for i in range(MG)]
                wtiles = {}
                for m in ms:
                    wt = wblk_pool.tile([PART, NK * PART], BF16, tag="wblk")
                    src = wqkv_blk[m].rearrange("p k c -> p (k c)")
                    hw = NK * PART // 2
                    nc.sync.dma_start(wt[:, 0:hw], src[:, 0:hw])
                    nc.sync.dma_start(wt[:, hw : 2 * hw], src[:, hw : 2 * hw])
                    wtiles[m] = wt
                pts = {}
                for m in ms:
                    pt = pq_pool.tile([PART, L], F32, tag="pq")
                    pts[m] = pt
                for k in range(NK):
                    while xt_loaded < min(NK, k + 2):
                        load_xt(xt_loaded)
                        xt_loaded += 1
                    for m in ms:
                        for nh in range(2):
                            nc.tensor.matmul(
                                pts[m][:, nh * 512 : (nh + 1) * 512],
                                wtiles[m][:, k * PART : (k + 1) * PART],
                                xt[k][:, nh * 512 : (nh + 1) * 512],
                                start=(k == 0),
                                stop=(k == NK - 1),
                            )
                for m in ms:
                    nc.scalar.activation(
                        qkvT[m][:],
                        pts[m][:],
                        mybir.ActivationFunctionType.Identity,
                        bias=bqkv_sb[:, m : m + 1],
                        scale=1.0,
                    )

        # ================= phase 2: attention per head pair =================
        with (
            tc.tile_pool(name="epool", bufs=6) as e_pool,
            tc.tile_pool(name="vtpool", bufs=4) as vt_pool,
            tc.tile_pool(name="otraw", bufs=6) as otraw_pool,
            tc.tile_pool(name="denp", bufs=2) as den_pool,
            tc.tile_pool(name="rcp", bufs=2) as rc_pool,
            tc.tile_pool(name="stp", bufs=2, space="PSUM") as st_pool,
            tc.tile_pool(name="pop", bufs=4, space="PSUM") as po_pool,
        ):
            def emit_vt_pair(j):
                """PE-transpose the full V pair tile (K=128) once; split the
                transposed chunks into the two heads' [128, 8*66] vt tiles
                (ones column appended for the denominator row)."""
                vsrc = qkvT[2 * NLQ + j]
                pvt = st_pool.tile([PART, L], BF16, tag="st")
                for c in range(NLK):
                    nc.tensor.transpose(
                        pvt[:, c * PART : (c + 1) * PART],
                        vsrc[:, c * PART : (c + 1) * PART],
                        ident[:],
                    )
                pvt3 = pvt[:].rearrange("p (c w) -> p c w", w=PART)
                out = []
                for side in range(2):
                    vt = vt_pool.tile([PART, NLK * VW], BF16, tag="vt")
                    nc.vector.memset(vt[:], 1.0)
                    vt3 = vt[:].rearrange("p (c w) -> p c w", w=VW)
                    nc.vector.tensor_copy(
                        vt3[:, :, 0:HD],
                        pvt3[:, :, side * HD : (side + 1) * HD],
                    )
                    out.append(vt)
                return out

            def emit_scores_exp(j, c):
                """Row-packed ST chunks for both sibling heads + fused exp."""
                ets = []
                st_e = st_pool.tile([PART, L], F32, tag="st")
                st_o = st_pool.tile([PART, L], F32, tag="st")
                sts = [st_e, st_o]
                # interleave row groups (0,64) so sibling heads' matmuls
                # overlap in the PE array (row tiling)
                for nh in range(2):
                    ns = slice(nh * 512, (nh + 1) * 512)
                    for side in range(2):
                        ro = side * HD
                        nc.tensor.matmul(
                            sts[side][:, ns],
                            qkvT[NLQ + j][ro : ro + HD, c * PART : (c + 1) * PART],
                            qkvT[j][ro : ro + HD, ns],
                            start=True,
                            stop=True,
                            tile_position=(ro, 0),
                        )
                for side in range(2):
                    et = e_pool.tile([PART, L], BF16, tag="e")
                    nc.scalar.activation(
                        et[:],
                        sts[side][:],
                        mybir.ActivationFunctionType.Exp,
                        bias=mb_sb[:, c : c + 1],
                        scale=1.0 / 8.0,
                    )
                    ets.append(et)
                return ets

            # batched softmax denominators: pair j's den rows ride in the
            # otr copies (bf16), get DMA-gathered into [8, 1024] batch
            # tiles (pairs 0-3 / 4-7), one cheap batched reciprocal each.
            den_bf = []
            rc_fr = []
            for bname in ("A", "B"):
                dbf = den_pool.tile([8, L], BF16, tag=f"dbf{bname}")
                den_bf.append(dbf)
                rfr = den_pool.tile([8, L], F32R, tag=f"rfr{bname}")
                rc_fr.append(rfr)

            def emit_batch_recip(b):
                dflt = den_pool.tile([8, L], F32, tag="dflt")
                rflt = den_pool.tile([8, L], F32, tag="rflt")
                nc.vector.tensor_copy(dflt[:], den_bf[b][:])
                with nc.allow_low_precision(reason="approx denom reciprocal"):
                    nc.vector.reciprocal_approx_fast(rflt[:], dflt[:])
                    nc.vector.tensor_copy(rc_fr[b][:], rflt[:])

            def flush_pending():
                pj, potr_e, potr_o = pending.pop(0)
                b, row = pj // 4, (pj % 4) * 2
                rc2 = rc_pool.tile([2, L], F32R, tag="rc")
                nc.sync.dma_start(rc2[0:1, :], rc_fr[b][row : row + 1, :])
                nc.sync.dma_start(rc2[1:2, :], rc_fr[b][row + 1 : row + 2, :])
                rt = st_pool.tile([PART, L], F32, tag="st")
                for half in range(2):
                    ns = slice(half * 512, (half + 1) * 512)
                    nc.tensor.matmul(
                        rt[:, ns], sel_sb[:], rc2[0:2, ns],
                        start=True, stop=True,
                    )
                nc.vector.tensor_mul(
                    ot_fin[pj][0:HD, :], potr_e[0:HD, :], rt[0:HD, :]
                )
                nc.vector.tensor_mul(
                    ot_fin[pj][HD:PART, :], potr_o[0:HD, :], rt[HD:PART, :]
                )

            pending = []
            # prologue: pair 0 V transpose + first score chunks
            vts = {0: emit_vt_pair(0)}
            ets_q = {0: [emit_scores_exp(0, 0)]}

            for j in range(NK):  # head pairs
                vt_e, vt_o = vts.pop(j)
                po_e0 = po_pool.tile([HD + 1, 512], F32, tag="po")
                po_e1 = po_pool.tile([HD + 1, 512], F32, tag="po")
                po_o0 = po_pool.tile([HD + 1, 512], F32, tag="po")
                po_o1 = po_pool.tile([HD + 1, 512], F32, tag="po")
                po = [[po_e0, po_e1], [po_o0, po_o1]]
                otr_e = otraw_pool.tile([HD + 1, L], BF16, tag="otre")
                otr_o = otraw_pool.tile([HD + 1, L], BF16, tag="otro")
                otrs = [otr_e, otr_o]

                for c in range(NLK):
                    # scores+exp for the NEXT chunk (this pair or next pair)
                    if c < NLK - 1:
                        ets_q[j].append(emit_scores_exp(j, c + 1))
                    else:
                        if j + 1 < NK:
                            vts[j + 1] = emit_vt_pair(j + 1)
                            ets_q[j + 1] = [emit_scores_exp(j + 1, 0)]
                    # pairs 0-3 normalize while pairs 5-7 compute
                    if j >= 5 and c in (2, 5) and pending and pending[0][0] < 4:
                        flush_pending()
                    et_pair = ets_q[j][c]
                    for side in range(2):
                        vt = vt_e if side == 0 else vt_o
                        for half in range(2):
                            ns = slice(half * 512, (half + 1) * 512)
                            nc.tensor.matmul(
                                po[side][half][:],
                                vt[:, c * VW : c * VW + HD + 1],
                                et_pair[side][:, ns],
                                start=(c == 0),
                                stop=(c == NLK - 1),
                            )
                del ets_q[j]

                # evacuate unnormalized outputs + denominator row (bf16)
                for side in range(2):
                    for half in range(2):
                        ns = slice(half * 512, (half + 1) * 512)
                        nc.vector.tensor_copy(
                            otrs[side][0 : HD + 1, ns], po[side][half][0 : HD + 1, :]
                        )
                    nc.sync.dma_start(
                        den_bf[j // 4][(j % 4) * 2 + side : (j % 4) * 2 + side + 1, :],
                        otrs[side][HD : HD + 1, :],
                    )
                pending.append((j, otr_e, otr_o))
                if j == 3 or j == NK - 1:
                    emit_batch_recip(j // 4)

            while pending:
                flush_pending()

        # ================= phase 3: output projection =================
        with (
            tc.tile_pool(name="woutp", bufs=1) as wout_pool,
            tc.tile_pool(name="fsb", bufs=2) as f_pool,
            tc.tile_pool(name="pf", bufs=2, space="PSUM") as pf_pool,
        ):
            bout_sb = wout_pool.tile([PART, D], BF16, tag="bout")
            nc.sync.dma_start(bout_sb[:], bout[:, :])
            wo = []
            for k in range(NK):
                t = wout_pool.tile([PART, D], BF16, tag=f"wo{k}")
                nc.sync.dma_start(t[:], wout[k * PART : (k + 1) * PART, :])
                wo.append(t)
            for lq in range(NLQ):
                pf = pf_pool.tile([PART, D], F32, tag="pf")
                for k in range(NK):
                    for nh in range(2):
                        nc.tensor.matmul(
                            pf[:, nh * 512 : (nh + 1) * 512],
                            ot_fin[k][:, lq * PART : (lq + 1) * PART],
                            wo[k][:, nh * 512 : (nh + 1) * 512],
                            start=(k == 0),
                            stop=(k == NK - 1),
                        )
                fs = f_pool.tile([PART, D], F32, tag="fsb")
                for half in range(2):
                    ns = slice(half * 512, (half + 1) * 512)
                    nc.vector.tensor_add(fs[:, ns], pf[:, ns], bout_sb[:, ns])
                    nc.sync.dma_start(Y[lq * PART : (lq + 1) * PART, ns], fs[:, ns])

    nc.compile()
    return nc


_NC_CACHE = None


def _get_nc():
    global _NC_CACHE
    if _NC_CACHE is None:
        _NC_CACHE = build_nc()
    return _NC_CACHE


def make_in_maps(x, attn_mask, w_qkv, b_qkv, w_out, b_out):
    """Host-side sharding + layout prep -> per-core input maps."""
    bf16 = ml_dtypes.bfloat16
    x = np.asarray(x, dtype=np.float32)
    attn_mask = np.asarray(attn_mask)
    w_qkv = np.asarray(w_qkv, dtype=np.float32)
    b_qkv = np.ascontiguousarray(np.asarray(b_qkv, dtype=np.float32))
    w_out = np.ascontiguousarray(np.asarray(w_out, dtype=np.float32).astype(bf16))
    b_out = np.asarray(b_out, dtype=np.float32).astype(bf16)

    # wqkv_blk[m, p, k, c] = w_qkv[k*128 + p, m*128 + c]
    wblk = np.ascontiguousarray(
        w_qkv.reshape(NK, PART, NM, PART).transpose(2, 1, 0, 3).astype(bf16)
    )
    maskbias = np.where(attn_mask.astype(bool), 0.0, -10000.0).astype(np.float32)

    sel_host = np.zeros((2, PART), dtype=np.float32)
    sel_host[0, 0:HD] = 1.0
    sel_host[1, HD:PART] = 1.0
    in_maps = []
    for b in range(B):
        in_maps.append(
            {
                "xT": np.ascontiguousarray(x[b].T.astype(bf16)),
                "wqkv_blk": wblk,
                "bqkv": b_qkv,
                "wout": w_out,
                "bout": np.ascontiguousarray(np.broadcast_to(b_out, (PART, D))),
                "maskb": np.ascontiguousarray(maskbias[b]),
                "sel": sel_host,
            }
        )
    return in_maps


def kernel(x, attn_mask, w_qkv, b_qkv, w_out, b_out):
    in_maps = make_in_maps(x, attn_mask, w_qkv, b_qkv, w_out, b_out)
    nc = _get_nc()
    res = run_bass_kernel_spmd(nc, in_maps, core_ids=list(range(N_CORES)))
    return np.stack([res.results[b]["Y"] for b in range(B)], axis=0)


if __name__ == "__main__":
    rng = np.random.default_rng(0)
    inputs = {
        "x": rng.standard_normal((B, L, D), dtype=np.float32),
        "attn_mask": np.ones((B, L), dtype=bool),
        "w_qkv": ((rng.random((D, D3), dtype=np.float32) - 0.5) / 16.0),
        "b_qkv": np.zeros((D3,), dtype=np.float32),
        "w_out": ((rng.random((D, D), dtype=np.float32) - 0.5) / 16.0),
        "b_out": np.zeros((D,), dtype=np.float32),
    }
    y = kernel(**inputs)
    print(y.shape, y.dtype)
